# revision 4
# baseline (speedup 1.0000x reference)
"""Trainium2 Bass kernel for DecoderAttentionRotary.

Problem: B=1, L=4096, D=1024, H=16 heads of d=64.
  qkv = x @ Wqkv + b; q,k get rotary embedding; causal attention per head.

Sharding: tensor parallel over heads — 8 cores x 2 heads each. Each core gets
the full (host-pre-transposed) activations plus its own column shard of Wqkv,
computes its 2 heads' attention output [L, 128] and the host concatenates.

Device-side layout choices:
  - x is fed pre-transposed (xT [D, L]) so the QKV projection produces
    q^T/k^T/v^T [128, L] directly (contraction dim on partitions).
  - Scores are computed transposed (S^T = K @ Q^T) so softmax probs come out
    in [k, q] layout, which is exactly the lhsT-free layout PV needs
    (out^T = [V|1]^T @ P^T accumulated over k blocks; the |1 column yields the
    softmax denominator for free).
  - RoPE pairs are laid out 16 partitions apart within 32-partition quadrants
    (via a host-side permutation of Wq/Wk columns) so the pair swap is a
    single DVE stream_shuffle.
  - matmuls run with float32r operand views: full PE rate at N>=256 while
    keeping ~fp32 precision.
"""

import sys

for _p in ("/opt/trn_rl_repo",):
    if _p not in sys.path:
        sys.path.insert(0, _p)

import numpy as np

import concourse.bass as bass
import concourse.mybir as mybir
import concourse.tile as tile
from concourse import bacc
from concourse import bass_utils
from concourse.masks import make_identity

F32 = mybir.dt.float32
F32R = mybir.dt.float32r
AFT = mybir.ActivationFunctionType

N_CORES = 8
NUM_HEADS = 16
HPC = NUM_HEADS // N_CORES  # heads per core = 2


class Cfg:
    def __init__(self, L=4096, D=1024, d=64, CH=3):
        self.L = L          # sequence length
        self.D = D          # model dim
        self.d = d          # head dim
        self.P = 128
        self.LB = 512       # projection l-block
        self.KB = 128       # key block
        self.QB = 512       # query block
        self.CH = CH        # k-blocks per exp chunk
        self.NLB = L // self.LB
        self.NKB = L // self.KB
        self.NQB = L // self.QB
        self.DK = D // self.P  # contraction tiles for projection


# Permutation of head-dim components: partition p (within a head's 64 rows)
# holds component comp(p).  Pairs (2i, 2i+1) end up 16 partitions apart inside
# one 32-partition quadrant, so stream_shuffle([16..31,0..15]) swaps pairs.
def _head_perm():
    perm = np.zeros(64, dtype=np.int64)
    for p in range(64):
        g, r = p // 32, p % 32
        perm[p] = 2 * (16 * g + (r % 16)) + (1 if r >= 16 else 0)
    return perm


_PERM = _head_perm()
_SWAP_MASK = [(i + 16) % 32 for i in range(32)]
_MASK_NEG = -1.0e30


def _build_program(cfg: Cfg):
    """Build (and bacc-compile) the per-core SPMD program."""
    P, L, d = cfg.P, cfg.L, cfg.d
    nc = bacc.Bacc(
        "TRN2",
        target_bir_lowering=False,
        debug=False,
        enable_asserts=False,
        num_devices=N_CORES,
    )

    xT_d = nc.dram_tensor("xT", [cfg.D, L], F32R, kind="ExternalInput")
    w_d = nc.dram_tensor("w", [cfg.D, 3 * HPC * d], F32R, kind="ExternalInput")
    b_d = nc.dram_tensor("b", [HPC * d, 3], F32, kind="ExternalInput")
    ropec_d = nc.dram_tensor("ropeC", [P, L], F32, kind="ExternalInput")
    ropes_d = nc.dram_tensor("ropeS", [P, L], F32, kind="ExternalInput")
    mask_d = nc.dram_tensor("mask", [P, P], F32, kind="ExternalInput")
    y_d = nc.dram_tensor("y", [HPC, L, d], F32, kind="ExternalOutput")

    scale = 1.0 / float(np.sqrt(d))

    with tile.TileContext(nc) as tc:
        with (
            tc.tile_pool(name="const", bufs=1) as const,
            tc.tile_pool(name="pers", bufs=1) as pers,
        ):
            ident = const.tile([P, P], F32, name="ident")
            make_identity(nc, ident)
            mask_sb = const.tile([P, P], F32, name="mask_sb")
            nc.sync.dma_start(mask_sb[:], mask_d.ap())
            b_sb = const.tile([HPC * d, 3], F32, name="b_sb")
            nc.sync.dma_start(b_sb[:], b_d.ap())
            w_sb = const.tile([P, cfg.DK, 3 * HPC * d], F32R, name="w_sb")
            nc.sync.dma_start(w_sb[:], w_d.ap().rearrange("(o p) c -> p o c", p=P))
            ropec = const.tile([P, L], F32, name="ropec")
            nc.sync.dma_start(ropec[:], ropec_d.ap())
            ropes = const.tile([P, L], F32, name="ropes")
            nc.sync.dma_start(ropes[:], ropes_d.ap())

            ones_f = const.tile([P, 1], F32, name="ones_f")
            nc.vector.memset(ones_f[:], 1.0)
            ones_r = const.tile([P, 1], F32R, name="ones_r")
            nc.vector.tensor_copy(ones_r[:], ones_f[:])

            # persistent transposed activations
            qR = pers.tile([P, L], F32R, name="qR")
            kR = pers.tile([P, L], F32R, name="kR")
            vT = pers.tile([P, L], F32, name="vT")
            # V in natural layout, with a ones column per head at col 64/65:
            # [p, kb, h, 66] ; lhsT slice for PV = vnat[:, kb, h, 0:65]
            vnat = pers.tile([P, cfg.NKB, HPC, 66], F32R, name="vnat")

            # ---------------- Phase A: QKV projection + RoPE ----------------
            with (
                tc.tile_pool(name="xtp", bufs=2) as xtp,
                tc.tile_pool(name="qkt", bufs=2) as qkt,
                tc.tile_pool(name="projp", bufs=3, space="PSUM") as pp,
            ):
                for lb in range(cfg.NLB):
                    ls = slice(lb * cfg.LB, (lb + 1) * cfg.LB)
                    xts = []
                    for dk in range(cfg.DK):
                        xt = xtp.tile([P, cfg.LB], F32R, name=f"xt{dk}", tag=f"xt{dk}")
                        nc.sync.dma_start(xt[:], xT_d.ap()[dk * P:(dk + 1) * P, ls])
                        xts.append(xt)
                    for t, dest in ((0, None), (1, None), (2, vT)):
                        ps = pp.tile([P, cfg.LB], F32, name="projps", tag="projps")
                        for dk in range(cfg.DK):
                            nc.tensor.matmul(
                                ps[:],
                                w_sb[:, dk, t * P:(t + 1) * P],
                                xts[dk][:],
                                start=(dk == 0),
                                stop=(dk == cfg.DK - 1),
                            )
                        if t == 2:
                            nc.scalar.activation(
                                vT[:, ls], ps[:], AFT.Identity,
                                bias=b_sb[:, 2:3], scale=1.0,
                            )
                        else:
                            # q/k: copy out with bias, then RoPE into qR/kR
                            raw = qkt.tile([P, cfg.LB], F32, name="qkraw", tag="qkraw")
                            nc.scalar.activation(
                                raw[:], ps[:], AFT.Identity,
                                bias=b_sb[:, t:t + 1], scale=1.0,
                            )
                            dst = qR if t == 0 else kR
                            sh = qkt.tile([P, cfg.LB], F32, name="ropesh", tag="ropesh")
                            nc.vector.stream_shuffle(sh[:], raw[:], _SWAP_MASK)
                            nc.vector.tensor_mul(sh[:], sh[:], ropes[:, ls])
                            tmp = qkt.tile([P, cfg.LB], F32, name="ropet", tag="ropet")
                            nc.vector.tensor_mul(tmp[:], raw[:], ropec[:, ls])
                            nc.vector.tensor_add(dst[:, ls], tmp[:], sh[:])

            # ---------------- Phase B: v^T -> V natural ----------------
            nc.vector.tensor_copy(
                vnat[:, :, :, 64:66],
                ones_r[:, None, None, :].to_broadcast((P, cfg.NKB, HPC, 2)),
            )
            with tc.tile_pool(name="vtp", bufs=2, space="PSUM") as tpp:
                for kb in range(cfg.NKB):
                    ps = tpp.tile([P, P], F32, name="vtps", tag="vtps")
                    nc.tensor.transpose(ps[:], vT[:, kb * P:(kb + 1) * P], ident[:])
                    nc.vector.tensor_copy(
                        vnat[:, kb, :, 0:64],
                        ps[:].rearrange("p (h c) -> p h c", c=64),
                    )

            # ---------------- Phase C: attention ----------------
            with (
                tc.tile_pool(name="qkp", bufs=2, space="PSUM") as qkp,
                tc.tile_pool(name="outp", bufs=1, space="PSUM") as op,
                tc.tile_pool(name="finp", bufs=1, space="PSUM") as fpp,
                tc.tile_pool(name="ptp", bufs=3) as ptp,
                tc.tile_pool(name="fin", bufs=2) as finp,
            ):
                for qb in range(cfg.NQB):
                    qs = slice(qb * cfg.QB, (qb + 1) * cfg.QB)
                    nkb = (qb + 1) * (cfg.QB // cfg.KB)
                    for hh in range(HPC):
                        hp = hh * d
                        outp = op.tile([65, cfg.QB], F32, name="outT", tag="outT")
                        for c0 in range(0, nkb, cfg.CH):
                            cn = min(cfg.CH, nkb - c0)
                            qk = qkp.tile(
                                [P, cfg.CH * cfg.QB], F32, name="qkps", tag="qkps"
                            )
                            for j in range(cn):
                                kb = c0 + j
                                nc.tensor.matmul(
                                    qk[:, j * cfg.QB:(j + 1) * cfg.QB],
                                    kR[hp:hp + d, kb * cfg.KB:(kb + 1) * cfg.KB],
                                    qR[hp:hp + d, qs],
                                    start=True,
                                    stop=True,
                                )
                                dd = kb - qb * (cfg.QB // cfg.KB)
                                if dd >= 0:
                                    lo = j * cfg.QB + dd * cfg.KB
                                    nc.vector.tensor_add(
                                        qk[:, lo:lo + cfg.KB],
                                        qk[:, lo:lo + cfg.KB],
                                        mask_sb[:],
                                    )
                            pt = ptp.tile(
                                [P, cfg.CH * cfg.QB], F32R, name="pt", tag="pt"
                            )
                            nc.scalar.activation(
                                pt[:, :cn * cfg.QB], qk[:, :cn * cfg.QB],
                                AFT.Exp, scale=scale,
                            )
                            for j in range(cn):
                                kb = c0 + j
                                dd = kb - qb * (cfg.QB // cfg.KB)
                                col0 = max(0, dd) * cfg.KB
                                nc.tensor.matmul(
                                    outp[:, col0:cfg.QB],
                                    vnat[:, kb, hh, 0:65],
                                    pt[:, j * cfg.QB + col0:(j + 1) * cfg.QB],
                                    start=(kb == 0),
                                    stop=(kb == nkb - 1),
                                )
                        # finalize: normalize + transpose to natural layout
                        os_ = finp.tile([65, cfg.QB], F32, name="osb", tag="osb")
                        nc.vector.tensor_copy(os_[:], outp[:])
                        for j in range(cfg.QB // P):
                            fps = fpp.tile([P, P], F32, name="finps", tag="finps")
                            nc.tensor.transpose(
                                fps[:, 0:65],
                                os_[:, j * P:(j + 1) * P],
                                ident[0:65, 0:65],
                            )
                            rec = finp.tile([P, 1], F32, name="rec", tag="rec")
                            nc.vector.reciprocal(rec[:], fps[:, 64:65])
                            of = finp.tile([P, d], F32, name="of", tag="of")
                            nc.vector.tensor_scalar_mul(of[:], fps[:, 0:64], rec[:])
                            l0 = (qb * (cfg.QB // P) + j) * P
                            nc.sync.dma_start(y_d.ap()[hh, l0:l0 + P, :], of[:])

    nc.compile()
    return nc


def _host_prep(cfg: Cfg, x, freqs_cis, Wqkv, bqkv):
    """Build the 8 per-core input maps (layout prep only, no math)."""
    P, L, D, d = cfg.P, cfg.L, cfg.D, cfg.d
    x = np.asarray(x, dtype=np.float32)
    freqs_cis = np.asarray(freqs_cis, dtype=np.float32)
    Wqkv = np.asarray(Wqkv, dtype=np.float32)
    bqkv = np.asarray(bqkv, dtype=np.float32)
    NH = D // d

    xT = np.ascontiguousarray(x.reshape(L, D).T)  # [D, L]

    Wq = Wqkv[:, 0:D].reshape(D, NH, d)
    Wk = Wqkv[:, D:2 * D].reshape(D, NH, d)
    Wv = Wqkv[:, 2 * D:3 * D].reshape(D, NH, d)
    bq = bqkv[0:D].reshape(NH, d)
    bk = bqkv[D:2 * D].reshape(NH, d)
    bv = bqkv[2 * D:3 * D].reshape(NH, d)

    cos = freqs_cis[:, :, 0]  # [L, d//2]
    sin = freqs_cis[:, :, 1]
    fidx = _PERM // 2                      # [64] frequency index per partition
    sgn = np.where(_PERM % 2 == 0, -1.0, 1.0).astype(np.float32)
    C_head = np.ascontiguousarray(cos[:, fidx].T)                    # [64, L]
    S_head = np.ascontiguousarray((sin[:, fidx] * sgn[None, :]).T)   # [64, L]
    ropeC = np.ascontiguousarray(np.concatenate([C_head] * HPC, axis=0))
    ropeS = np.ascontiguousarray(np.concatenate([S_head] * HPC, axis=0))

    ii = np.arange(P)
    mask = np.where(ii[None, :] >= ii[:, None], 0.0, _MASK_NEG).astype(np.float32)

    in_maps = []
    for c in range(N_CORES):
        heads = [HPC * c + i for i in range(HPC)]
        wq = np.concatenate([Wq[:, h, :][:, _PERM] for h in heads], axis=1)
        wk = np.concatenate([Wk[:, h, :][:, _PERM] for h in heads], axis=1)
        wv = np.concatenate([Wv[:, h, :] for h in heads], axis=1)
        w_core = np.ascontiguousarray(
            np.concatenate([wq, wk, wv], axis=1))            # [D, 384]
        b_core = np.ascontiguousarray(np.stack(
            [
                np.concatenate([bq[h][_PERM] for h in heads]),
                np.concatenate([bk[h][_PERM] for h in heads]),
                np.concatenate([bv[h] for h in heads]),
            ],
            axis=1,
        ).astype(np.float32))                                # [128, 3]
        in_maps.append({
            "xT": xT,
            "w": w_core,
            "b": b_core,
            "ropeC": ropeC,
            "ropeS": ropeS,
            "mask": mask,
        })
    return in_maps


_PROG_CACHE = {}


def _get_program(cfg: Cfg):
    key = (cfg.L, cfg.D, cfg.d, cfg.CH)
    if key not in _PROG_CACHE:
        _PROG_CACHE[key] = _build_program(cfg)
    return _PROG_CACHE[key]


def kernel(x, freqs_cis, Wqkv, bqkv, _trace=False):
    cfg = Cfg()
    nc = _get_program(cfg)
    in_maps = _host_prep(cfg, x, freqs_cis, Wqkv, bqkv)
    res = bass_utils.run_bass_kernel_spmd(
        nc, in_maps, core_ids=list(range(N_CORES)), trace=_trace,
    )
    out = np.empty((cfg.L, cfg.D), dtype=np.float32)
    for c in range(N_CORES):
        y = res.results[c]["y"]  # [HPC, L, d]
        for hh in range(HPC):
            h = HPC * c + hh
            out[:, h * cfg.d:(h + 1) * cfg.d] = y[hh]
    kernel._last_results = res
    return out.reshape(1, cfg.L, cfg.D)


# revision 5
# speedup vs baseline: 1.0231x; 1.0231x over previous
"""Trainium2 Bass kernel for DecoderAttentionRotary.

Problem: B=1, L=4096, D=1024, H=16 heads of d=64.
  qkv = x @ Wqkv + b; q,k get rotary embedding; causal attention per head.

Sharding: tensor parallel over heads — 8 cores x 2 heads each. Each core gets
the full (host-pre-transposed) activations plus its own column shard of Wqkv,
computes its 2 heads' attention output [L, 128] and the host concatenates.

Device-side layout choices:
  - x is fed pre-transposed (xT [D, L]) so the QKV projection produces
    q^T/k^T/v^T [128, L] directly (contraction dim on partitions).
  - Scores are computed transposed (S^T = K @ Q^T) so softmax probs come out
    in [k, q] layout, which is exactly the lhsT-free layout PV needs
    (out^T = [V|1]^T @ P^T accumulated over k blocks; the |1 column yields the
    softmax denominator for free).
  - RoPE pairs are laid out 16 partitions apart within 32-partition quadrants
    (via a host-side permutation of Wq/Wk columns) so the pair swap is a
    single DVE stream_shuffle.
  - matmuls run with float32r operand views: full PE rate at N>=256 while
    keeping ~fp32 precision.
"""

import sys

for _p in ("/opt/trn_rl_repo",):
    if _p not in sys.path:
        sys.path.insert(0, _p)

import numpy as np

import concourse.bass as bass
import concourse.mybir as mybir
import concourse.tile as tile
from concourse import bacc
from concourse import bass_utils
from concourse.masks import make_identity

F32 = mybir.dt.float32
F32R = mybir.dt.float32r
AFT = mybir.ActivationFunctionType

N_CORES = 8
NUM_HEADS = 16
HPC = NUM_HEADS // N_CORES  # heads per core = 2


class Cfg:
    def __init__(self, L=4096, D=1024, d=64, CH=3):
        self.L = L          # sequence length
        self.D = D          # model dim
        self.d = d          # head dim
        self.P = 128
        self.LB = 512       # projection l-block
        self.KB = 128       # key block
        self.QB = 512       # query block
        self.CH = CH        # k-blocks per exp chunk
        self.NLB = L // self.LB
        self.NKB = L // self.KB
        self.NQB = L // self.QB
        self.DK = D // self.P  # contraction tiles for projection


# Permutation of head-dim components: partition p (within a head's 64 rows)
# holds component comp(p).  Pairs (2i, 2i+1) end up 16 partitions apart inside
# one 32-partition quadrant, so stream_shuffle([16..31,0..15]) swaps pairs.
def _head_perm():
    perm = np.zeros(64, dtype=np.int64)
    for p in range(64):
        g, r = p // 32, p % 32
        perm[p] = 2 * (16 * g + (r % 16)) + (1 if r >= 16 else 0)
    return perm


_PERM = _head_perm()
_SWAP_MASK = [(i + 16) % 32 for i in range(32)]
_MASK_NEG = -1.0e30


def _build_program(cfg: Cfg, nrep: int = 1):
    """Build (and bacc-compile) the per-core SPMD program.

    nrep>1 wraps the whole body in a hardware For_i loop (benchmark mode:
    one dispatch runs the kernel nrep times so device time is measurable
    above the axon dispatch floor)."""
    P, L, d = cfg.P, cfg.L, cfg.d
    nc = bacc.Bacc(
        "TRN2",
        target_bir_lowering=False,
        debug=False,
        enable_asserts=False,
        num_devices=N_CORES,
    )

    xT_d = nc.dram_tensor("xT", [cfg.D, L], F32R, kind="ExternalInput")
    w_d = nc.dram_tensor("w", [cfg.D, 3 * HPC * d], F32R, kind="ExternalInput")
    b_d = nc.dram_tensor("b", [HPC * d, 3], F32, kind="ExternalInput")
    ropec_d = nc.dram_tensor("ropeC", [P, L], F32, kind="ExternalInput")
    ropes_d = nc.dram_tensor("ropeS", [P, L], F32, kind="ExternalInput")
    mask_d = nc.dram_tensor("mask", [P, P], F32, kind="ExternalInput")
    y_d = nc.dram_tensor("y", [HPC, L, d], F32, kind="ExternalOutput")

    scale = 1.0 / float(np.sqrt(d))

    import contextlib

    with tile.TileContext(nc) as tc:
        rep_ctx = tc.For_i(0, nrep, 1) if nrep > 1 else contextlib.nullcontext()
        with (
            rep_ctx,
            tc.tile_pool(name="const", bufs=1) as const,
            tc.tile_pool(name="pers", bufs=1) as pers,
        ):
            ident = const.tile([P, P], F32, name="ident")
            make_identity(nc, ident)
            mask_sb = const.tile([P, P], F32, name="mask_sb")
            nc.sync.dma_start(mask_sb[:], mask_d.ap())
            b_sb = const.tile([HPC * d, 3], F32, name="b_sb")
            nc.sync.dma_start(b_sb[:], b_d.ap())
            w_sb = const.tile([P, cfg.DK, 3 * HPC * d], F32R, name="w_sb")
            nc.sync.dma_start(w_sb[:], w_d.ap().rearrange("(o p) c -> p o c", p=P))
            ropec = const.tile([P, L], F32, name="ropec")
            nc.sync.dma_start(ropec[:], ropec_d.ap())
            ropes = const.tile([P, L], F32, name="ropes")
            nc.sync.dma_start(ropes[:], ropes_d.ap())

            ones_f = const.tile([P, 1], F32, name="ones_f")
            nc.vector.memset(ones_f[:], 1.0)
            ones_r = const.tile([P, 1], F32R, name="ones_r")
            nc.vector.tensor_copy(ones_r[:], ones_f[:])

            # persistent transposed activations
            qR = pers.tile([P, L], F32R, name="qR")
            kR = pers.tile([P, L], F32R, name="kR")
            vT = pers.tile([P, L], F32, name="vT")
            # V in natural layout, with a ones column per head at col 64/65:
            # [p, kb, h, 66] ; lhsT slice for PV = vnat[:, kb, h, 0:65]
            vnat = pers.tile([P, cfg.NKB, HPC, 66], F32R, name="vnat")

            # ---------------- Phase A: QKV projection + RoPE ----------------
            with (
                tc.tile_pool(name="xtp", bufs=2) as xtp,
                tc.tile_pool(name="qkt", bufs=2) as qkt,
                tc.tile_pool(name="projp", bufs=3, space="PSUM") as pp,
            ):
                for lb in range(cfg.NLB):
                    ls = slice(lb * cfg.LB, (lb + 1) * cfg.LB)
                    xts = []
                    for dk in range(cfg.DK):
                        xt = xtp.tile([P, cfg.LB], F32R, name=f"xt{dk}", tag=f"xt{dk}")
                        nc.sync.dma_start(xt[:], xT_d.ap()[dk * P:(dk + 1) * P, ls])
                        xts.append(xt)
                    for t, dest in ((0, None), (1, None), (2, vT)):
                        ps = pp.tile([P, cfg.LB], F32, name="projps", tag="projps")
                        for dk in range(cfg.DK):
                            nc.tensor.matmul(
                                ps[:],
                                w_sb[:, dk, t * P:(t + 1) * P],
                                xts[dk][:],
                                start=(dk == 0),
                                stop=(dk == cfg.DK - 1),
                            )
                        if t == 2:
                            nc.scalar.activation(
                                vT[:, ls], ps[:], AFT.Identity,
                                bias=b_sb[:, 2:3], scale=1.0,
                            )
                        else:
                            # q/k: copy out with bias, then RoPE into qR/kR
                            raw = qkt.tile([P, cfg.LB], F32, name="qkraw", tag="qkraw")
                            nc.scalar.activation(
                                raw[:], ps[:], AFT.Identity,
                                bias=b_sb[:, t:t + 1], scale=1.0,
                            )
                            dst = qR if t == 0 else kR
                            sh = qkt.tile([P, cfg.LB], F32, name="ropesh", tag="ropesh")
                            nc.vector.stream_shuffle(sh[:], raw[:], _SWAP_MASK)
                            nc.vector.tensor_mul(sh[:], sh[:], ropes[:, ls])
                            tmp = qkt.tile([P, cfg.LB], F32, name="ropet", tag="ropet")
                            nc.vector.tensor_mul(tmp[:], raw[:], ropec[:, ls])
                            nc.vector.tensor_add(dst[:, ls], tmp[:], sh[:])

            # ---------------- Phase B: v^T -> V natural ----------------
            nc.vector.tensor_copy(
                vnat[:, :, :, 64:66],
                ones_r[:, None, None, :].to_broadcast((P, cfg.NKB, HPC, 2)),
            )
            with tc.tile_pool(name="vtp", bufs=2, space="PSUM") as tpp:
                for kb in range(cfg.NKB):
                    ps = tpp.tile([P, P], F32, name="vtps", tag="vtps")
                    nc.tensor.transpose(ps[:], vT[:, kb * P:(kb + 1) * P], ident[:])
                    nc.vector.tensor_copy(
                        vnat[:, kb, :, 0:64],
                        ps[:].rearrange("p (h c) -> p h c", c=64),
                    )

            # ---------------- Phase C: attention ----------------
            with (
                tc.tile_pool(name="qkp", bufs=2, space="PSUM") as qkp,
                tc.tile_pool(name="outp", bufs=1, space="PSUM") as op,
                tc.tile_pool(name="finp", bufs=1, space="PSUM") as fpp,
                tc.tile_pool(name="ptp", bufs=3) as ptp,
                tc.tile_pool(name="fin", bufs=2) as finp,
            ):
                for qb in range(cfg.NQB):
                    qs = slice(qb * cfg.QB, (qb + 1) * cfg.QB)
                    nkb = (qb + 1) * (cfg.QB // cfg.KB)
                    for hh in range(HPC):
                        hp = hh * d
                        outp = op.tile([65, cfg.QB], F32, name="outT", tag="outT")
                        for c0 in range(0, nkb, cfg.CH):
                            cn = min(cfg.CH, nkb - c0)
                            qk = qkp.tile(
                                [P, cfg.CH * cfg.QB], F32, name="qkps", tag="qkps"
                            )
                            for j in range(cn):
                                kb = c0 + j
                                nc.tensor.matmul(
                                    qk[:, j * cfg.QB:(j + 1) * cfg.QB],
                                    kR[hp:hp + d, kb * cfg.KB:(kb + 1) * cfg.KB],
                                    qR[hp:hp + d, qs],
                                    start=True,
                                    stop=True,
                                )
                                dd = kb - qb * (cfg.QB // cfg.KB)
                                if dd >= 0:
                                    lo = j * cfg.QB + dd * cfg.KB
                                    nc.vector.tensor_add(
                                        qk[:, lo:lo + cfg.KB],
                                        qk[:, lo:lo + cfg.KB],
                                        mask_sb[:],
                                    )
                            pt = ptp.tile(
                                [P, cfg.CH * cfg.QB], F32R, name="pt", tag="pt"
                            )
                            nc.scalar.activation(
                                pt[:, :cn * cfg.QB], qk[:, :cn * cfg.QB],
                                AFT.Exp, scale=scale,
                            )
                            for j in range(cn):
                                kb = c0 + j
                                dd = kb - qb * (cfg.QB // cfg.KB)
                                col0 = max(0, dd) * cfg.KB
                                nc.tensor.matmul(
                                    outp[:, col0:cfg.QB],
                                    vnat[:, kb, hh, 0:65],
                                    pt[:, j * cfg.QB + col0:(j + 1) * cfg.QB],
                                    start=(kb == 0),
                                    stop=(kb == nkb - 1),
                                )
                        # finalize: normalize + transpose to natural layout
                        os_ = finp.tile([65, cfg.QB], F32, name="osb", tag="osb")
                        nc.vector.tensor_copy(os_[:], outp[:])
                        for j in range(cfg.QB // P):
                            fps = fpp.tile([P, P], F32, name="finps", tag="finps")
                            nc.tensor.transpose(
                                fps[:, 0:65],
                                os_[:, j * P:(j + 1) * P],
                                ident[0:65, 0:65],
                            )
                            rec = finp.tile([P, 1], F32, name="rec", tag="rec")
                            nc.vector.reciprocal(rec[:], fps[:, 64:65])
                            of = finp.tile([P, d], F32, name="of", tag="of")
                            nc.vector.tensor_scalar_mul(of[:], fps[:, 0:64], rec[:])
                            l0 = (qb * (cfg.QB // P) + j) * P
                            nc.sync.dma_start(y_d.ap()[hh, l0:l0 + P, :], of[:])

    nc.compile()
    return nc


def _host_prep(cfg: Cfg, x, freqs_cis, Wqkv, bqkv):
    """Build the 8 per-core input maps (layout prep only, no math)."""
    P, L, D, d = cfg.P, cfg.L, cfg.D, cfg.d
    x = np.asarray(x, dtype=np.float32)
    freqs_cis = np.asarray(freqs_cis, dtype=np.float32)
    Wqkv = np.asarray(Wqkv, dtype=np.float32)
    bqkv = np.asarray(bqkv, dtype=np.float32)
    NH = D // d

    xT = np.ascontiguousarray(x.reshape(L, D).T)  # [D, L]

    Wq = Wqkv[:, 0:D].reshape(D, NH, d)
    Wk = Wqkv[:, D:2 * D].reshape(D, NH, d)
    Wv = Wqkv[:, 2 * D:3 * D].reshape(D, NH, d)
    bq = bqkv[0:D].reshape(NH, d)
    bk = bqkv[D:2 * D].reshape(NH, d)
    bv = bqkv[2 * D:3 * D].reshape(NH, d)

    cos = freqs_cis[:, :, 0]  # [L, d//2]
    sin = freqs_cis[:, :, 1]
    fidx = _PERM // 2                      # [64] frequency index per partition
    sgn = np.where(_PERM % 2 == 0, -1.0, 1.0).astype(np.float32)
    C_head = np.ascontiguousarray(cos[:, fidx].T)                    # [64, L]
    S_head = np.ascontiguousarray((sin[:, fidx] * sgn[None, :]).T)   # [64, L]
    ropeC = np.ascontiguousarray(np.concatenate([C_head] * HPC, axis=0))
    ropeS = np.ascontiguousarray(np.concatenate([S_head] * HPC, axis=0))

    ii = np.arange(P)
    mask = np.where(ii[None, :] >= ii[:, None], 0.0, _MASK_NEG).astype(np.float32)

    in_maps = []
    for c in range(N_CORES):
        heads = [HPC * c + i for i in range(HPC)]
        wq = np.concatenate([Wq[:, h, :][:, _PERM] for h in heads], axis=1)
        wk = np.concatenate([Wk[:, h, :][:, _PERM] for h in heads], axis=1)
        wv = np.concatenate([Wv[:, h, :] for h in heads], axis=1)
        w_core = np.ascontiguousarray(
            np.concatenate([wq, wk, wv], axis=1))            # [D, 384]
        b_core = np.ascontiguousarray(np.stack(
            [
                np.concatenate([bq[h][_PERM] for h in heads]),
                np.concatenate([bk[h][_PERM] for h in heads]),
                np.concatenate([bv[h] for h in heads]),
            ],
            axis=1,
        ).astype(np.float32))                                # [128, 3]
        in_maps.append({
            "xT": xT,
            "w": w_core,
            "b": b_core,
            "ropeC": ropeC,
            "ropeS": ropeS,
            "mask": mask,
        })
    return in_maps


_PROG_CACHE = {}


def _get_program(cfg: Cfg, nrep: int = 1):
    key = (cfg.L, cfg.D, cfg.d, cfg.CH, nrep)
    if key not in _PROG_CACHE:
        _PROG_CACHE[key] = _build_program(cfg, nrep=nrep)
    return _PROG_CACHE[key]


def kernel(x, freqs_cis, Wqkv, bqkv, _trace=False):
    cfg = Cfg()
    nc = _get_program(cfg)
    in_maps = _host_prep(cfg, x, freqs_cis, Wqkv, bqkv)
    res = bass_utils.run_bass_kernel_spmd(
        nc, in_maps, core_ids=list(range(N_CORES)), trace=_trace,
    )
    out = np.empty((cfg.L, cfg.D), dtype=np.float32)
    for c in range(N_CORES):
        y = res.results[c]["y"]  # [HPC, L, d]
        for hh in range(HPC):
            h = HPC * c + hh
            out[:, h * cfg.d:(h + 1) * cfg.d] = y[hh]
    kernel._last_results = res
    return out.reshape(1, cfg.L, cfg.D)


# revision 8
# speedup vs baseline: 271.6514x; 265.5196x over previous
"""Trainium2 Bass kernel for DecoderAttentionRotary.

Problem: B=1, L=4096, D=1024, H=16 heads of d=64.
  qkv = x @ Wqkv + b; q,k get rotary embedding; causal attention per head.

Sharding: tensor parallel over heads — 8 cores x 2 heads each. Each core gets
the full (host-pre-transposed) activations plus its own column shard of Wqkv,
computes its 2 heads' attention output [L, 128] and the host concatenates.

Device-side layout choices:
  - x is fed pre-transposed (xT [D, L]) so the QKV projection produces
    q^T/k^T/v^T [128, L] directly (contraction dim on partitions).
  - Scores are computed transposed (S^T = K @ Q^T) so softmax probs come out
    in [k, q] layout, which is exactly the lhsT-free layout PV needs
    (out^T = [V|1]^T @ P^T accumulated over k blocks; the |1 column yields the
    softmax denominator for free).
  - RoPE pairs are laid out 16 partitions apart within 32-partition quadrants
    (via a host-side permutation of Wq/Wk columns) so the pair swap is a
    single DVE stream_shuffle.
  - matmuls run with float32r operand views: full PE rate at N>=256 while
    keeping ~fp32 precision.
"""

import sys

for _p in ("/opt/trn_rl_repo",):
    if _p not in sys.path:
        sys.path.insert(0, _p)

import numpy as np

import concourse.bass as bass
import concourse.mybir as mybir
import concourse.tile as tile
from concourse import bacc
from concourse import bass_utils
from concourse.masks import make_identity

F32 = mybir.dt.float32
F32R = mybir.dt.float32r
AFT = mybir.ActivationFunctionType

N_CORES = 8
NUM_HEADS = 16
HPC = NUM_HEADS // N_CORES  # heads per core = 2


class Cfg:
    def __init__(self, L=4096, D=1024, d=64, CH=3):
        self.L = L          # sequence length
        self.D = D          # model dim
        self.d = d          # head dim
        self.P = 128
        self.LB = 512       # projection l-block
        self.KB = 128       # key block
        self.QB = 512       # query block
        self.CH = CH        # k-blocks per exp chunk
        self.NLB = L // self.LB
        self.NKB = L // self.KB
        self.NQB = L // self.QB
        self.DK = D // self.P  # contraction tiles for projection


# Permutation of head-dim components: partition p (within a head's 64 rows)
# holds component comp(p).  Pairs (2i, 2i+1) end up 16 partitions apart inside
# one 32-partition quadrant, so stream_shuffle([16..31,0..15]) swaps pairs.
def _head_perm():
    perm = np.zeros(64, dtype=np.int64)
    for p in range(64):
        g, r = p // 32, p % 32
        perm[p] = 2 * (16 * g + (r % 16)) + (1 if r >= 16 else 0)
    return perm


_PERM = _head_perm()
_SWAP_MASK = [(i + 16) % 32 for i in range(32)]
_MASK_NEG = -1.0e30


def _build_program(cfg: Cfg, nrep: int = 1):
    """Build (and bacc-compile) the per-core SPMD program.

    nrep>1 wraps the whole body in a hardware For_i loop (benchmark mode:
    one dispatch runs the kernel nrep times so device time is measurable
    above the axon dispatch floor)."""
    P, L, d = cfg.P, cfg.L, cfg.d
    nc = bacc.Bacc(
        "TRN2",
        target_bir_lowering=False,
        debug=False,
        enable_asserts=False,
        num_devices=N_CORES,
    )

    xT_d = nc.dram_tensor("xT", [cfg.D, L], F32R, kind="ExternalInput")
    w_d = nc.dram_tensor("w", [cfg.D, 3 * HPC * d], F32R, kind="ExternalInput")
    b_d = nc.dram_tensor("b", [HPC * d, 3], F32, kind="ExternalInput")
    ropec_d = nc.dram_tensor("ropeC", [P, L], F32, kind="ExternalInput")
    ropes_d = nc.dram_tensor("ropeS", [P, L], F32, kind="ExternalInput")
    mask_d = nc.dram_tensor("mask", [P, P], F32, kind="ExternalInput")
    y_d = nc.dram_tensor("y", [HPC, d, L], F32, kind="ExternalOutput")

    scale = 1.0 / float(np.sqrt(d))

    import contextlib

    with tile.TileContext(nc) as tc:
        rep_ctx = tc.For_i(0, nrep, 1) if nrep > 1 else contextlib.nullcontext()
        with (
            rep_ctx,
            tc.tile_pool(name="const", bufs=1) as const,
            tc.tile_pool(name="pers", bufs=1) as pers,
        ):
            ident = const.tile([P, P], F32, name="ident")
            make_identity(nc, ident)
            mask_sb = const.tile([P, P], F32, name="mask_sb")
            nc.sync.dma_start(mask_sb[:], mask_d.ap())
            b_sb = const.tile([HPC * d, 3], F32, name="b_sb")
            nc.sync.dma_start(b_sb[:], b_d.ap())
            w_sb = const.tile([P, cfg.DK, 3 * HPC * d], F32R, name="w_sb")
            nc.sync.dma_start(w_sb[:], w_d.ap().rearrange("(o p) c -> p o c", p=P))
            ropec = const.tile([P, L], F32, name="ropec")
            nc.sync.dma_start(ropec[:], ropec_d.ap())
            ropes = const.tile([P, L], F32, name="ropes")
            nc.sync.dma_start(ropes[:], ropes_d.ap())

            ones_f = const.tile([P, 1], F32, name="ones_f")
            nc.vector.memset(ones_f[:], 1.0)
            ones_r = const.tile([P, 1], F32R, name="ones_r")
            nc.vector.tensor_copy(ones_r[:], ones_f[:])

            # persistent transposed activations
            qR = pers.tile([P, L], F32R, name="qR")
            kR = pers.tile([P, L], F32R, name="kR")
            vT = pers.tile([P, L], F32, name="vT")
            # V in natural layout, with a ones column per head at col 64/65:
            # [p, kb, h, 66] ; lhsT slice for PV = vnat[:, kb, h, 0:65]
            vnat = pers.tile([P, cfg.NKB, HPC, 66], F32R, name="vnat")

            # ---------------- Phase A: QKV projection + RoPE ----------------
            with (
                tc.tile_pool(name="xtp", bufs=2) as xtp,
                tc.tile_pool(name="qkt", bufs=2) as qkt,
                tc.tile_pool(name="projp", bufs=3, space="PSUM") as pp,
            ):
                for lb in range(cfg.NLB):
                    ls = slice(lb * cfg.LB, (lb + 1) * cfg.LB)
                    xts = []
                    for dk in range(cfg.DK):
                        xt = xtp.tile([P, cfg.LB], F32R, name=f"xt{dk}", tag=f"xt{dk}")
                        nc.sync.dma_start(xt[:], xT_d.ap()[dk * P:(dk + 1) * P, ls])
                        xts.append(xt)
                    for t, dest in ((0, None), (1, None), (2, vT)):
                        ps = pp.tile([P, cfg.LB], F32, name="projps", tag="projps")
                        for dk in range(cfg.DK):
                            nc.tensor.matmul(
                                ps[:],
                                w_sb[:, dk, t * P:(t + 1) * P],
                                xts[dk][:],
                                start=(dk == 0),
                                stop=(dk == cfg.DK - 1),
                            )
                        if t == 2:
                            nc.vector.tensor_scalar_add(
                                vT[:, ls], ps[:], b_sb[:, 2:3])
                        else:
                            # q/k: copy out with bias, then RoPE into qR/kR
                            raw = qkt.tile([P, cfg.LB], F32, name="qkraw", tag="qkraw")
                            nc.vector.tensor_scalar_add(
                                raw[:], ps[:], b_sb[:, t:t + 1])
                            dst = qR if t == 0 else kR
                            sh = qkt.tile([P, cfg.LB], F32, name="ropesh", tag="ropesh")
                            nc.vector.stream_shuffle(sh[:], raw[:], _SWAP_MASK)
                            nc.vector.tensor_mul(sh[:], sh[:], ropes[:, ls])
                            tmp = qkt.tile([P, cfg.LB], F32, name="ropet", tag="ropet")
                            nc.vector.tensor_mul(tmp[:], raw[:], ropec[:, ls])
                            nc.vector.tensor_add(dst[:, ls], tmp[:], sh[:])

            # ---------------- Phase B: v^T -> V natural ----------------
            nc.vector.tensor_copy(
                vnat[:, :, :, 64:66],
                ones_r[:, None, None, :].to_broadcast((P, cfg.NKB, HPC, 2)),
            )
            with tc.tile_pool(name="vtp", bufs=2, space="PSUM") as tpp:
                for kb in range(cfg.NKB):
                    ps = tpp.tile([P, P], F32, name="vtps", tag="vtps")
                    nc.tensor.transpose(ps[:], vT[:, kb * P:(kb + 1) * P], ident[:])
                    nc.vector.tensor_copy(
                        vnat[:, kb, :, 0:64],
                        ps[:].rearrange("p (h c) -> p h c", c=64),
                    )

            # ---------------- Phase C: attention ----------------
            # Heads interleaved per k-block: the two K=64 QK matmuls sit in
            # adjacent row-groups (tile_position (0,0)/(64,0)) and run
            # concurrently on the PE.  Scores/probs for both heads live in one
            # [128, 2, 512] tile (2 PSUM banks) so exp covers both heads in a
            # single ACTIVATE.  Output stays transposed ([d, L]); the softmax
            # denominator sits in partition 64 of the PV accumulator and is
            # divided out with a partition-broadcast multiply.  Host undoes
            # the transpose.
            with (
                tc.tile_pool(name="qkp", bufs=3, space="PSUM") as qkp,
                tc.tile_pool(name="outp", bufs=1, space="PSUM") as op,
                tc.tile_pool(name="ptp", bufs=3) as ptp,
                tc.tile_pool(name="nrm", bufs=2) as nrm,
            ):
                NB = cfg.QB // cfg.KB
                for qb in range(cfg.NQB):
                    nkb = (qb + 1) * NB
                    outs = [
                        op.tile([65, cfg.QB], F32, name=f"outT{hh}", tag=f"outT{hh}")
                        for hh in range(HPC)
                    ]
                    for kb in range(nkb):
                        dd = kb - qb * NB
                        col0 = max(0, dd) * cfg.KB
                        qk = qkp.tile([P, HPC, cfg.QB], F32, name="qkps", tag="qkps")
                        for hh in range(HPC):
                            hp = hh * d
                            nc.tensor.matmul(
                                qk[:, hh, col0:cfg.QB],
                                kR[hp:hp + d, kb * cfg.KB:(kb + 1) * cfg.KB],
                                qR[hp:hp + d, qb * cfg.QB + col0:(qb + 1) * cfg.QB],
                                start=True,
                                stop=True,
                            )
                        if dd >= 0:
                            for hh in range(HPC):
                                nc.vector.tensor_add(
                                    qk[:, hh, col0:col0 + cfg.KB],
                                    qk[:, hh, col0:col0 + cfg.KB],
                                    mask_sb[:],
                                )
                        pt = ptp.tile([P, HPC, cfg.QB], F32R, name="pt", tag="pt")
                        nc.scalar.activation(
                            pt[:, :, col0:cfg.QB], qk[:, :, col0:cfg.QB],
                            AFT.Exp, scale=scale,
                        )
                        for hh in range(HPC):
                            nc.tensor.matmul(
                                outs[hh][:, col0:cfg.QB],
                                vnat[:, kb, hh, 0:65],
                                pt[:, hh, col0:cfg.QB],
                                start=(kb == 0),
                                stop=(kb == nkb - 1),
                            )
                    # normalize in transposed layout and store [d, qb-block]
                    for hh in range(HPC):
                        rec = nrm.tile([1, cfg.QB], F32, name="rec", tag="rec")
                        nc.vector.reciprocal(rec[:], outs[hh][64:65, :])
                        recb = nrm.tile([d, cfg.QB], F32, name="recb", tag="recb")
                        nc.gpsimd.partition_broadcast(recb[:], rec[:], d)
                        yt = nrm.tile([d, cfg.QB], F32, name="yt", tag="yt")
                        nc.vector.tensor_mul(yt[:], outs[hh][0:d, :], recb[:])
                        nc.sync.dma_start(
                            y_d.ap()[hh, :, qb * cfg.QB:(qb + 1) * cfg.QB], yt[:])

    nc.compile()
    return nc


def _host_prep(cfg: Cfg, x, freqs_cis, Wqkv, bqkv):
    """Build the 8 per-core input maps (layout prep only, no math)."""
    P, L, D, d = cfg.P, cfg.L, cfg.D, cfg.d
    x = np.asarray(x, dtype=np.float32)
    freqs_cis = np.asarray(freqs_cis, dtype=np.float32)
    Wqkv = np.asarray(Wqkv, dtype=np.float32)
    bqkv = np.asarray(bqkv, dtype=np.float32)
    NH = D // d

    xT = np.ascontiguousarray(x.reshape(L, D).T)  # [D, L]

    Wq = Wqkv[:, 0:D].reshape(D, NH, d)
    Wk = Wqkv[:, D:2 * D].reshape(D, NH, d)
    Wv = Wqkv[:, 2 * D:3 * D].reshape(D, NH, d)
    bq = bqkv[0:D].reshape(NH, d)
    bk = bqkv[D:2 * D].reshape(NH, d)
    bv = bqkv[2 * D:3 * D].reshape(NH, d)

    cos = freqs_cis[:, :, 0]  # [L, d//2]
    sin = freqs_cis[:, :, 1]
    fidx = _PERM // 2                      # [64] frequency index per partition
    sgn = np.where(_PERM % 2 == 0, -1.0, 1.0).astype(np.float32)
    C_head = np.ascontiguousarray(cos[:, fidx].T)                    # [64, L]
    S_head = np.ascontiguousarray((sin[:, fidx] * sgn[None, :]).T)   # [64, L]
    ropeC = np.ascontiguousarray(np.concatenate([C_head] * HPC, axis=0))
    ropeS = np.ascontiguousarray(np.concatenate([S_head] * HPC, axis=0))

    ii = np.arange(P)
    mask = np.where(ii[None, :] >= ii[:, None], 0.0, _MASK_NEG).astype(np.float32)

    in_maps = []
    for c in range(N_CORES):
        heads = [HPC * c + i for i in range(HPC)]
        wq = np.concatenate([Wq[:, h, :][:, _PERM] for h in heads], axis=1)
        wk = np.concatenate([Wk[:, h, :][:, _PERM] for h in heads], axis=1)
        wv = np.concatenate([Wv[:, h, :] for h in heads], axis=1)
        w_core = np.ascontiguousarray(
            np.concatenate([wq, wk, wv], axis=1))            # [D, 384]
        b_core = np.ascontiguousarray(np.stack(
            [
                np.concatenate([bq[h][_PERM] for h in heads]),
                np.concatenate([bk[h][_PERM] for h in heads]),
                np.concatenate([bv[h] for h in heads]),
            ],
            axis=1,
        ).astype(np.float32))                                # [128, 3]
        in_maps.append({
            "xT": xT,
            "w": w_core,
            "b": b_core,
            "ropeC": ropeC,
            "ropeS": ropeS,
            "mask": mask,
        })
    return in_maps


_PROG_CACHE = {}


def _get_program(cfg: Cfg, nrep: int = 1):
    key = (cfg.L, cfg.D, cfg.d, cfg.CH, nrep)
    if key not in _PROG_CACHE:
        _PROG_CACHE[key] = _build_program(cfg, nrep=nrep)
    return _PROG_CACHE[key]


def kernel(x, freqs_cis, Wqkv, bqkv, _trace=False):
    cfg = Cfg()
    nc = _get_program(cfg)
    in_maps = _host_prep(cfg, x, freqs_cis, Wqkv, bqkv)
    res = bass_utils.run_bass_kernel_spmd(
        nc, in_maps, core_ids=list(range(N_CORES)), trace=_trace,
    )
    out = np.empty((cfg.L, cfg.D), dtype=np.float32)
    for c in range(N_CORES):
        y = res.results[c]["y"]  # [HPC, d, L]
        for hh in range(HPC):
            h = HPC * c + hh
            out[:, h * cfg.d:(h + 1) * cfg.d] = y[hh].T
    kernel._last_results = res
    return out.reshape(1, cfg.L, cfg.D)


# revision 9
# speedup vs baseline: 306.8480x; 1.1296x over previous
"""Trainium2 Bass kernel for DecoderAttentionRotary.

Problem: B=1, L=4096, D=1024, H=16 heads of d=64.
  qkv = x @ Wqkv + b; q,k get rotary embedding; causal attention per head.

Sharding: tensor parallel over heads — 8 cores x 2 heads each. Each core gets
the full (host-pre-transposed) activations plus its own column shard of Wqkv,
computes its 2 heads' attention output [L, 128] and the host concatenates.

Device-side layout choices:
  - x is fed pre-transposed (xT [D, L]) so the QKV projection produces
    q^T/k^T/v^T [128, L] directly (contraction dim on partitions).
  - Scores are computed transposed (S^T = K @ Q^T) so softmax probs come out
    in [k, q] layout, which is exactly the lhsT-free layout PV needs
    (out^T = [V|1]^T @ P^T accumulated over k blocks; the |1 column yields the
    softmax denominator for free).
  - RoPE pairs are laid out 16 partitions apart within 32-partition quadrants
    (via a host-side permutation of Wq/Wk columns) so the pair swap is a
    single DVE stream_shuffle.
  - matmuls run with float32r operand views: full PE rate at N>=256 while
    keeping ~fp32 precision.
"""

import sys

for _p in ("/opt/trn_rl_repo",):
    if _p not in sys.path:
        sys.path.insert(0, _p)

import numpy as np

import concourse.bass as bass
import concourse.mybir as mybir
import concourse.tile as tile
from concourse import bacc
from concourse import bass_utils
from concourse.masks import make_identity

F32 = mybir.dt.float32
F32R = mybir.dt.float32r
AFT = mybir.ActivationFunctionType

N_CORES = 8
NUM_HEADS = 16
HPC = NUM_HEADS // N_CORES  # heads per core = 2


class Cfg:
    def __init__(self, L=4096, D=1024, d=64, CH=3):
        self.L = L          # sequence length
        self.D = D          # model dim
        self.d = d          # head dim
        self.P = 128
        self.LB = 512       # projection l-block
        self.KB = 128       # key block
        self.QB = 512       # query block
        self.CH = CH        # k-blocks per exp chunk
        self.NLB = L // self.LB
        self.NKB = L // self.KB
        self.NQB = L // self.QB
        self.DK = D // self.P  # contraction tiles for projection


# Permutation of head-dim components: partition p (within a head's 64 rows)
# holds component comp(p).  Pairs (2i, 2i+1) end up 16 partitions apart inside
# one 32-partition quadrant, so stream_shuffle([16..31,0..15]) swaps pairs.
def _head_perm():
    perm = np.zeros(64, dtype=np.int64)
    for p in range(64):
        g, r = p // 32, p % 32
        perm[p] = 2 * (16 * g + (r % 16)) + (1 if r >= 16 else 0)
    return perm


_PERM = _head_perm()
_SWAP_MASK = [(i + 16) % 32 for i in range(32)]
_MASK_NEG = -1.0e30


def _build_program(cfg: Cfg, nrep: int = 1):
    """Build (and bacc-compile) the per-core SPMD program.

    nrep>1 wraps the whole body in a hardware For_i loop (benchmark mode:
    one dispatch runs the kernel nrep times so device time is measurable
    above the axon dispatch floor)."""
    P, L, d = cfg.P, cfg.L, cfg.d
    nc = bacc.Bacc(
        "TRN2",
        target_bir_lowering=False,
        debug=False,
        enable_asserts=False,
        num_devices=N_CORES,
    )

    xT_d = nc.dram_tensor("xT", [cfg.D, L], F32R, kind="ExternalInput")
    w_d = nc.dram_tensor("w", [cfg.D, 3 * HPC * d], F32R, kind="ExternalInput")
    b_d = nc.dram_tensor("b", [HPC * d, 3], F32, kind="ExternalInput")
    ropec_d = nc.dram_tensor("ropeC", [P, L], F32, kind="ExternalInput")
    ropes_d = nc.dram_tensor("ropeS", [P, L], F32, kind="ExternalInput")
    mask_d = nc.dram_tensor("mask", [P, P], F32, kind="ExternalInput")
    y_d = nc.dram_tensor("y", [HPC, d, L], F32, kind="ExternalOutput")

    scale = 1.0 / float(np.sqrt(d))

    import contextlib

    with tile.TileContext(nc) as tc:
        rep_ctx = tc.For_i(0, nrep, 1) if nrep > 1 else contextlib.nullcontext()
        with (
            rep_ctx,
            tc.tile_pool(name="const", bufs=1) as const,
            tc.tile_pool(name="pers", bufs=1) as pers,
        ):
            ident = const.tile([P, P], F32, name="ident")
            make_identity(nc, ident)
            mask_sb = const.tile([P, P], F32, name="mask_sb")
            nc.sync.dma_start(mask_sb[:], mask_d.ap())
            b_sb = const.tile([HPC * d, 3], F32, name="b_sb")
            nc.sync.dma_start(b_sb[:], b_d.ap())
            w_sb = const.tile([P, cfg.DK, 3 * HPC * d], F32R, name="w_sb")
            nc.sync.dma_start(w_sb[:], w_d.ap().rearrange("(o p) c -> p o c", p=P))
            ropec = const.tile([P, L], F32, name="ropec")
            nc.sync.dma_start(ropec[:], ropec_d.ap())
            ropes = const.tile([P, L], F32, name="ropes")
            nc.sync.dma_start(ropes[:], ropes_d.ap())

            ones_f = const.tile([P, 1], F32, name="ones_f")
            nc.vector.memset(ones_f[:], 1.0)
            ones_r = const.tile([P, 1], F32R, name="ones_r")
            nc.vector.tensor_copy(ones_r[:], ones_f[:])

            # persistent transposed activations
            qR = pers.tile([P, L], F32R, name="qR")
            kR = pers.tile([P, L], F32R, name="kR")
            vT = pers.tile([P, L], F32, name="vT")
            # V in natural layout, with a ones column per head at col 64/65:
            # [p, kb, h, 66] ; lhsT slice for PV = vnat[:, kb, h, 0:65]
            vnat = pers.tile([P, cfg.NKB, HPC, 66], F32R, name="vnat")

            # ---------------- Phase A: QKV projection + RoPE ----------------
            with (
                tc.tile_pool(name="xtp", bufs=2) as xtp,
                tc.tile_pool(name="qkt", bufs=2) as qkt,
                tc.tile_pool(name="projp", bufs=3, space="PSUM") as pp,
            ):
                for lb in range(cfg.NLB):
                    ls = slice(lb * cfg.LB, (lb + 1) * cfg.LB)
                    xts = []
                    for dk in range(cfg.DK):
                        xt = xtp.tile([P, cfg.LB], F32R, name=f"xt{dk}", tag=f"xt{dk}")
                        nc.sync.dma_start(xt[:], xT_d.ap()[dk * P:(dk + 1) * P, ls])
                        xts.append(xt)
                    for t, dest in ((0, None), (1, None), (2, vT)):
                        ps = pp.tile([P, cfg.LB], F32, name="projps", tag="projps")
                        for dk in range(cfg.DK):
                            nc.tensor.matmul(
                                ps[:],
                                w_sb[:, dk, t * P:(t + 1) * P],
                                xts[dk][:],
                                start=(dk == 0),
                                stop=(dk == cfg.DK - 1),
                            )
                        if t == 2:
                            nc.vector.tensor_scalar_add(
                                vT[:, ls], ps[:], b_sb[:, 2:3])
                        else:
                            # q/k: copy out with bias, then RoPE into qR/kR
                            raw = qkt.tile([P, cfg.LB], F32, name="qkraw", tag="qkraw")
                            nc.vector.tensor_scalar_add(
                                raw[:], ps[:], b_sb[:, t:t + 1])
                            dst = qR if t == 0 else kR
                            sh = qkt.tile([P, cfg.LB], F32, name="ropesh", tag="ropesh")
                            nc.vector.stream_shuffle(sh[:], raw[:], _SWAP_MASK)
                            nc.vector.tensor_mul(sh[:], sh[:], ropes[:, ls])
                            tmp = qkt.tile([P, cfg.LB], F32, name="ropet", tag="ropet")
                            nc.vector.tensor_mul(tmp[:], raw[:], ropec[:, ls])
                            nc.vector.tensor_add(dst[:, ls], tmp[:], sh[:])

            # ---------------- Phase B: v^T -> V natural ----------------
            nc.vector.tensor_copy(
                vnat[:, :, :, 64:66],
                ones_r[:, None, None, :].to_broadcast((P, cfg.NKB, HPC, 2)),
            )
            with tc.tile_pool(name="vtp", bufs=2, space="PSUM") as tpp:
                for kb in range(cfg.NKB):
                    ps = tpp.tile([P, P], F32, name="vtps", tag="vtps")
                    nc.tensor.transpose(ps[:], vT[:, kb * P:(kb + 1) * P], ident[:])
                    nc.vector.tensor_copy(
                        vnat[:, kb, :, 0:64],
                        ps[:].rearrange("p (h c) -> p h c", c=64),
                    )

            # ---------------- Phase C: attention ----------------
            # Heads interleaved per k-block: the two K=64 QK matmuls sit in
            # adjacent row-groups (tile_position (0,0)/(64,0)) and run
            # concurrently on the PE.  Scores/probs for both heads live in one
            # [128, 2, 512] tile (2 PSUM banks) so exp covers both heads in a
            # single ACTIVATE.  Output stays transposed ([d, L]); the softmax
            # denominator sits in partition 64 of the PV accumulator and is
            # divided out with a partition-broadcast multiply.  Host undoes
            # the transpose.
            with (
                tc.tile_pool(name="qkp", bufs=3, space="PSUM") as qkp,
                tc.tile_pool(name="outp", bufs=1, space="PSUM") as op,
                tc.tile_pool(name="ptp", bufs=3) as ptp,
                tc.tile_pool(name="nrm", bufs=2) as nrm,
            ):
                NB = cfg.QB // cfg.KB
                for qb in range(cfg.NQB):
                    nkb = (qb + 1) * NB
                    outs = [
                        op.tile([65, cfg.QB], F32, name=f"outT{hh}", tag=f"outT{hh}")
                        for hh in range(HPC)
                    ]
                    for kb in range(nkb):
                        dd = kb - qb * NB
                        col0 = max(0, dd) * cfg.KB
                        qk = qkp.tile([P, HPC, cfg.QB], F32, name="qkps", tag="qkps")
                        for hh in range(HPC):
                            hp = hh * d
                            nc.tensor.matmul(
                                qk[:, hh, col0:cfg.QB],
                                kR[hp:hp + d, kb * cfg.KB:(kb + 1) * cfg.KB],
                                qR[hp:hp + d, qb * cfg.QB + col0:(qb + 1) * cfg.QB],
                                start=True,
                                stop=True,
                            )
                        if dd >= 0:
                            nc.vector.tensor_add(
                                qk[:, :, col0:col0 + cfg.KB],
                                qk[:, :, col0:col0 + cfg.KB],
                                mask_sb[:, None, :].to_broadcast(
                                    (P, HPC, cfg.KB)),
                            )
                        pt = ptp.tile([P, HPC, cfg.QB], F32R, name="pt", tag="pt")
                        nc.scalar.activation(
                            pt[:, :, col0:cfg.QB], qk[:, :, col0:cfg.QB],
                            AFT.Exp, scale=scale,
                        )
                        for hh in range(HPC):
                            nc.tensor.matmul(
                                outs[hh][:, col0:cfg.QB],
                                vnat[:, kb, hh, 0:65],
                                pt[:, hh, col0:cfg.QB],
                                start=(kb == 0),
                                stop=(kb == nkb - 1),
                            )
                    # normalize in transposed layout and store [d, qb-block]
                    for hh in range(HPC):
                        rec = nrm.tile([1, cfg.QB], F32, name="rec", tag="rec")
                        nc.vector.reciprocal(rec[:], outs[hh][64:65, :])
                        recb = nrm.tile([d, cfg.QB], F32, name="recb", tag="recb")
                        nc.gpsimd.partition_broadcast(recb[:], rec[:], d)
                        yt = nrm.tile([d, cfg.QB], F32, name="yt", tag="yt")
                        nc.vector.tensor_mul(yt[:], outs[hh][0:d, :], recb[:])
                        nc.sync.dma_start(
                            y_d.ap()[hh, :, qb * cfg.QB:(qb + 1) * cfg.QB], yt[:])

    nc.compile()
    return nc


def _host_prep(cfg: Cfg, x, freqs_cis, Wqkv, bqkv):
    """Build the 8 per-core input maps (layout prep only, no math)."""
    P, L, D, d = cfg.P, cfg.L, cfg.D, cfg.d
    x = np.asarray(x, dtype=np.float32)
    freqs_cis = np.asarray(freqs_cis, dtype=np.float32)
    Wqkv = np.asarray(Wqkv, dtype=np.float32)
    bqkv = np.asarray(bqkv, dtype=np.float32)
    NH = D // d

    xT = np.ascontiguousarray(x.reshape(L, D).T)  # [D, L]

    Wq = Wqkv[:, 0:D].reshape(D, NH, d)
    Wk = Wqkv[:, D:2 * D].reshape(D, NH, d)
    Wv = Wqkv[:, 2 * D:3 * D].reshape(D, NH, d)
    bq = bqkv[0:D].reshape(NH, d)
    bk = bqkv[D:2 * D].reshape(NH, d)
    bv = bqkv[2 * D:3 * D].reshape(NH, d)

    cos = freqs_cis[:, :, 0]  # [L, d//2]
    sin = freqs_cis[:, :, 1]
    fidx = _PERM // 2                      # [64] frequency index per partition
    sgn = np.where(_PERM % 2 == 0, -1.0, 1.0).astype(np.float32)
    C_head = np.ascontiguousarray(cos[:, fidx].T)                    # [64, L]
    S_head = np.ascontiguousarray((sin[:, fidx] * sgn[None, :]).T)   # [64, L]
    ropeC = np.ascontiguousarray(np.concatenate([C_head] * HPC, axis=0))
    ropeS = np.ascontiguousarray(np.concatenate([S_head] * HPC, axis=0))

    ii = np.arange(P)
    mask = np.where(ii[None, :] >= ii[:, None], 0.0, _MASK_NEG).astype(np.float32)

    in_maps = []
    for c in range(N_CORES):
        heads = [HPC * c + i for i in range(HPC)]
        wq = np.concatenate([Wq[:, h, :][:, _PERM] for h in heads], axis=1)
        wk = np.concatenate([Wk[:, h, :][:, _PERM] for h in heads], axis=1)
        wv = np.concatenate([Wv[:, h, :] for h in heads], axis=1)
        w_core = np.ascontiguousarray(
            np.concatenate([wq, wk, wv], axis=1))            # [D, 384]
        b_core = np.ascontiguousarray(np.stack(
            [
                np.concatenate([bq[h][_PERM] for h in heads]),
                np.concatenate([bk[h][_PERM] for h in heads]),
                np.concatenate([bv[h] for h in heads]),
            ],
            axis=1,
        ).astype(np.float32))                                # [128, 3]
        in_maps.append({
            "xT": xT,
            "w": w_core,
            "b": b_core,
            "ropeC": ropeC,
            "ropeS": ropeS,
            "mask": mask,
        })
    return in_maps


_PROG_CACHE = {}


def _get_program(cfg: Cfg, nrep: int = 1):
    key = (cfg.L, cfg.D, cfg.d, cfg.CH, nrep)
    if key not in _PROG_CACHE:
        _PROG_CACHE[key] = _build_program(cfg, nrep=nrep)
    return _PROG_CACHE[key]


def kernel(x, freqs_cis, Wqkv, bqkv, _trace=False):
    cfg = Cfg()
    nc = _get_program(cfg)
    in_maps = _host_prep(cfg, x, freqs_cis, Wqkv, bqkv)
    res = bass_utils.run_bass_kernel_spmd(
        nc, in_maps, core_ids=list(range(N_CORES)), trace=_trace,
    )
    out = np.empty((cfg.L, cfg.D), dtype=np.float32)
    for c in range(N_CORES):
        y = res.results[c]["y"]  # [HPC, d, L]
        for hh in range(HPC):
            h = HPC * c + hh
            out[:, h * cfg.d:(h + 1) * cfg.d] = y[hh].T
    kernel._last_results = res
    return out.reshape(1, cfg.L, cfg.D)
